# revision 17
# baseline (speedup 1.0000x reference)
"""Trainium2 Bass kernel for nn_CopyStack (copy-mechanism vocab scatter).

Computes, for full inputs:
    enc   = tanh(encoder_outputs @ W_proj + b_proj)          [B,S,H]
    score = decoder_outputs @ enc^T + input_bias             [B,T,S]
    probs = softmax(score, axis=-1)                          [B,T,S]
    out[b,t,v] = sum_{s: inputs[b,s]==v} probs[b,t,s]        [B,T,V]

Sharding: 8 cores = (batch b, vocab half vh). Core c = 2*b + vh produces
out[b, :, vh*16000 : vh*16000+16000] directly in [T, V] layout.

Pipeline (v5): W (sync ring) and eT (scalar ring) stream-feed the
k-major enc matmuls; dT and host-built one-hot masks ride the sync ring
behind W. The host merges duplicate token ids into single slots via a
multi-hot permute matrix (psl). input_bias is folded into the scores
accumulation as a K=1 ones@bias f32r matmul; exp reads scores straight
from PSUM. The scatter is hybrid: windows 5-7 are produced by GpSimd
local_scatter (slot values + int16 offsets -> zeroed 2000-wide rows, no
PSUM evictions), windows 0-4 by one-hot mask matmuls with PSUM->SBUF
evictions alternating vector/scalar. All three compute engines plus the
PE produce output concurrently; per-window DMAs on three rings overlap
production. Windows are 2000 wide (16000 columns). Output fp16.
"""

import numpy as np

import concourse.bacc as bacc
import concourse.tile as tile
from concourse import mybir
from concourse.bass_utils import run_bass_kernel_spmd
from concourse.masks import make_identity

F32 = mybir.dt.float32
F32R = mybir.dt.float32r
F16 = mybir.dt.float16
I16 = mybir.dt.int16

B, S, T, H, V = 4, 512, 256, 1024, 32000
N_CORES = 8
KH = H // 128         # 8 hidden chunks
KS = S // 128         # 4 source chunks
TC2 = T // 128        # 2 target chunks
WIN = 2000            # vocab window width
NW = 8                # windows per core (8 * 2000 = 16000)
NWM = 5               # windows 0..4 via mask matmul; 5..7 via local_scatter
CAP = 128             # slot capacity per window
VH = 16000            # vocab columns owned per core
SLW = 500             # window matmul slice width


def build_bass():
    nc = bacc.Bacc()

    w = nc.dram_tensor("w", [H, H], F32R, kind="ExternalInput")        # W_proj
    eT = nc.dram_tensor("eT", [H, S], F32R, kind="ExternalInput")      # E[b]^T
    dT = nc.dram_tensor("dT", [H, T], F32R, kind="ExternalInput")      # D[b]^T
    bproj = nc.dram_tensor("bproj", [H], F32, kind="ExternalInput")
    sbias = nc.dram_tensor("sbias", [S], F32R, kind="ExternalInput")  # input_bias[b]
    ones = nc.dram_tensor("ones", [1, 128], F32R, kind="ExternalInput")
    maskh = nc.dram_tensor("maskh", [128, NWM * WIN], F16, kind="ExternalInput")
    pslh = nc.dram_tensor("pslh", [128, KS * NW * CAP], F16, kind="ExternalInput")
    idxh = nc.dram_tensor("idxh", [128, NW * CAP], I16, kind="ExternalInput")

    out16 = nc.dram_tensor("out16", [T, VH], F16, kind="ExternalOutput")

    with tile.TileContext(nc) as tc:
        with (
            tc.tile_pool(name="big", bufs=1) as big,
            tc.tile_pool(name="work", bufs=1) as work,
            tc.tile_pool(name="outp", bufs=4) as outp,
        ):
            # ---- sync ring: W chunks (critical), then dT, then masks ----
            w_t = []
            for k in range(KH):
                wt = big.tile([128, H], F32R, tag=f"w{k}", name=f"w{k}")
                nc.sync.dma_start(wt[:], w[k * 128:(k + 1) * 128, :])
                w_t.append(wt)
            dT_sb = big.tile([128, KH * T], F32R, tag="dT", name="dT")
            nc.sync.dma_start(
                dT_sb[:].rearrange("p (c t) -> p c t", c=KH),
                dT[:, :].rearrange("(c p) t -> p c t", p=128))
            mask_sb = big.tile([128, NWM * WIN], F16, tag="maskh", name="maskh")
            for g in range(NWM):
                nc.sync.dma_start(
                    mask_sb[:, g * WIN:(g + 1) * WIN],
                    maskh[:, g * WIN:(g + 1) * WIN])

            # ---- scalar ring: eT chunks only (second critical stream) ----
            eT_t = []
            for k in range(KH):
                et = big.tile([128, S], F32R, tag=f"eT{k}", name=f"eT{k}")
                nc.scalar.dma_start(et[:], eT[k * 128:(k + 1) * 128, :])
                eT_t.append(et)

            # ---- gpsimd ring: small tensors (early, cheap) ----
            psl_sb = big.tile([128, KS * NW * CAP], F16, tag="pslh", name="pslh")
            nc.gpsimd.dma_start(psl_sb[:], pslh[:, :])
            idx_sb = work.tile([128, NW * CAP], I16, tag="idxh")
            nc.gpsimd.dma_start(idx_sb[:], idxh[:, :])
            bproj_sb = work.tile([128, KH], F32, tag="bproj")
            nc.gpsimd.dma_start(bproj_sb[:], bproj[:].rearrange("(c p) -> p c", p=128))
            sbias_row = work.tile([1, S], F32R, tag="sbias_row")
            nc.gpsimd.dma_start(
                sbias_row[:], sbias[:].rearrange("(o s) -> o s", o=1))
            ones1 = work.tile([1, 128], F32R, tag="ones1")
            nc.gpsimd.dma_start(ones1[:], ones[:, :])

            scratch = work.tile([128, S], F32, tag="scratch")
            nc.vector.memset(scratch[:], 0.25)
            ident16 = work.tile([128, 128], F16, tag="ident16")
            make_identity(nc, ident16[:])

            # ---- PE warm-up: ramp the clock before W lands ----
            with tc.tile_pool(name="warm", bufs=2, space="PSUM") as wp:
                wt0 = wp.tile([128, S], F32, tag="w0", name="warm0")
                wt1 = wp.tile([128, S], F32, tag="w1", name="warm1")
                for i in range(12):
                    nc.tensor.matmul(
                        (wt0 if i % 2 == 0 else wt1)[:],
                        lhsT=scratch[:, :128], rhs=scratch[:],
                        start=True, stop=True,
                    )

            encT = []
            for m in range(KH):
                et = big.tile([128, S], F32R, tag=f"encT{m}", name=f"encT{m}")
                encT.append(et)
            probsT = []
            for sc in range(KS):
                t_ = work.tile([128, T], F16, tag=f"probsT{sc}",
                               name=f"probsT{sc}")
                probsT.append(t_)

            with tc.tile_pool(name="acc8", bufs=1, space="PSUM") as acc8:
                # ---- enc: k-major streaming over W/eT chunk arrivals ----
                pm = {}
                for m in range(KH):
                    pm[m] = acc8.tile([128, S], F32, tag=f"pm{m}",
                                      name=f"pm{m}")
                for k in range(KH):
                    for m in range(KH):
                        nc.tensor.matmul(
                            pm[m][:],
                            lhsT=w_t[k][:, m * 128:(m + 1) * 128],
                            rhs=eT_t[k][:],
                            start=(k == 0), stop=(k == KH - 1),
                        )
                for m in range(KH):
                    nc.scalar.activation(
                        encT[m][:], pm[m][:],
                        mybir.ActivationFunctionType.Tanh,
                        bias=bproj_sb[:, m:m + 1], scale=1.0,
                    )

            probs_l = []
            with (
                tc.tile_pool(name="sc", bufs=2, space="PSUM") as scp,
                tc.tile_pool(name="tp", bufs=2, space="PSUM") as tpp,
            ):
                # ---- scores (bias via K=1 ones@sbias matmul), softmax.
                #      tc0/tc1 interleaved per-m so the PE paces with the
                #      serial tanh chain on the scalar engine. ----
                ps_l = [scp.tile([128, S], F32, tag="ps", name=f"ps{t_}")
                        for t_ in range(TC2)]
                for tc_i in range(TC2):
                    nc.tensor.matmul(
                        ps_l[tc_i][:], lhsT=ones1[:], rhs=sbias_row[:],
                        start=True, stop=False,
                    )
                for m in range(KH):
                    for tc_i in range(TC2):
                        nc.tensor.matmul(
                            ps_l[tc_i][:],
                            lhsT=dT_sb[:, m * T + tc_i * 128:
                                       m * T + (tc_i + 1) * 128],
                            rhs=encT[m][:],
                            start=False, stop=(m == KH - 1),
                        )
                for tc_i in range(TC2):
                    ps = ps_l[tc_i]
                    rmax = work.tile([128, 1], F32, tag=f"rmax{tc_i}",
                                     name=f"rmax{tc_i}")
                    nc.vector.reduce_max(rmax[:], ps[:],
                                         axis=mybir.AxisListType.X)
                    nrmax = work.tile([128, 1], F32, tag=f"nrmax{tc_i}",
                                      name=f"nrmax{tc_i}")
                    nc.vector.tensor_scalar_mul(nrmax[:], rmax[:], -1.0)
                    ex = work.tile([128, S], F32, tag=f"ex{tc_i}",
                                   name=f"ex{tc_i}")
                    rsum = work.tile([128, 1], F32, tag=f"rsum{tc_i}",
                                     name=f"rsum{tc_i}")
                    nc.scalar.activation(
                        ex[:], ps[:], mybir.ActivationFunctionType.Exp,
                        bias=nrmax[:, :1], scale=1.0, accum_out=rsum[:, :1],
                    )
                    rinv = work.tile([128, 1], F32, tag=f"rinv{tc_i}",
                                     name=f"rinv{tc_i}")
                    nc.vector.reciprocal(rinv[:], rsum[:])
                    probs = work.tile([128, S], F16, tag=f"probs{tc_i}",
                                      name=f"probs{tc_i}")
                    nc.vector.tensor_scalar_mul(probs[:], ex[:], rinv[:, :1])
                    probs_l.append(probs)

                # ---- probs^T via PE f16 transposes, DVE evictions ----
                for tc_i in range(TC2):
                    probs = probs_l[tc_i]
                    for sc in range(KS):
                        pt = tpp.tile([128, 128], F16, tag="tp",
                                      name=f"pt{tc_i}_{sc}")
                        nc.tensor.transpose(
                            out=pt[:], in_=probs[:, sc * 128:(sc + 1) * 128],
                            identity=ident16[:],
                        )
                        nc.vector.tensor_copy(
                            probsT[sc][:, tc_i * 128:(tc_i + 1) * 128],
                            pt[:])

            # ---- scatter ----
            out_v = out16[:, :].rearrange("(c p) j -> p c j", p=128)

            def permute(pjp, wi):
                pj = pjp.tile([128, T], F32, tag="pj", name=f"pj{wi}")
                for sc in range(KS):
                    nc.tensor.matmul(
                        pj[:],
                        lhsT=psl_sb[:, sc * NW * CAP + wi * CAP:
                                    sc * NW * CAP + (wi + 1) * CAP],
                        rhs=probsT[sc][:],
                        start=(sc == 0), stop=(sc == KS - 1),
                    )
                return pj

            with (
                tc.tile_pool(name="pj", bufs=2, space="PSUM") as pjp,
                tc.tile_pool(name="tp2", bufs=2, space="PSUM") as tp2,
                tc.tile_pool(name="so", bufs=4, space="PSUM") as sop,
            ):
                # -- local_scatter windows first: feed GpSimd early --
                obs = {}
                for wi in range(NWM, NW):
                    pj = permute(pjp, wi)
                    pws = work.tile([128, T], F16, tag=f"pws{wi}",
                                    name=f"pws{wi}")
                    nc.vector.tensor_copy(pws[:], pj[:])
                    ob = outp.tile([128, TC2 * WIN], F16, tag="ob",
                                   name=f"ob{wi}")
                    obs[wi] = ob
                    for tc_i in range(TC2):
                        ptw = tp2.tile([128, 128], F16, tag="tp2",
                                       name=f"ptw{wi}_{tc_i}")
                        nc.tensor.transpose(
                            out=ptw[:], in_=pws[:, tc_i * 128:(tc_i + 1) * 128],
                            identity=ident16[:],
                        )
                        pwT = work.tile([128, 128], F16, tag=f"pwT{wi}_{tc_i}",
                                        name=f"pwT{wi}_{tc_i}")
                        nc.vector.tensor_copy(pwT[:], ptw[:])
                        nc.gpsimd.local_scatter(
                            out_ap=ob[:, tc_i * WIN:(tc_i + 1) * WIN],
                            data_ap=pwT[:],
                            idxs_ap=idx_sb[:, wi * CAP:(wi + 1) * CAP],
                            channels=128, num_elems=WIN, num_idxs=CAP,
                        )
                    nc.gpsimd.dma_start(
                        out_v[:, :, wi * WIN:(wi + 1) * WIN],
                        ob[:].rearrange("p (c j) -> p c j", c=TC2),
                    )

                # -- mask-matmul windows: PE + vector/scalar evictions --
                evict = (nc.vector.tensor_copy, nc.scalar.copy)
                ei = 0
                for wi in range(NWM):
                    pj = permute(pjp, wi)
                    pw = work.tile([128, T], F16, tag=f"pslotT{wi}",
                                   name=f"pslotT{wi}")
                    evict[ei % 2](pw[:], pj[:])
                    ei += 1
                    ob = outp.tile([128, TC2 * WIN], F16, tag="ob",
                                   name=f"ob{wi}")
                    for tc_i in range(TC2):
                        for n in range(4):
                            po = sop.tile([128, SLW], F32, tag="so",
                                          name=f"po{wi}_{tc_i}_{n}")
                            nc.tensor.matmul(
                                po[:],
                                lhsT=pw[:, tc_i * 128:(tc_i + 1) * 128],
                                rhs=mask_sb[:, wi * WIN + n * SLW:
                                            wi * WIN + (n + 1) * SLW],
                                start=True, stop=True,
                            )
                            dst = ob[:, tc_i * WIN + n * SLW:
                                     tc_i * WIN + (n + 1) * SLW]
                            evict[ei % 2](dst, po[:])
                            ei += 1
                    eng = (nc.sync, nc.scalar)[wi % 2]
                    eng.dma_start(
                        out_v[:, :, wi * WIN:(wi + 1) * WIN],
                        ob[:].rearrange("p (c j) -> p c j", c=TC2),
                    )

    nc.finalize()
    return nc


_NC_CACHE = None


def _get_nc():
    global _NC_CACHE
    if _NC_CACHE is None:
        _NC_CACHE = build_bass()
    return _NC_CACHE


def _prep_slots(ids_b: np.ndarray, lo: int):
    """Bucket tokens into NW windows of WIN columns, merging duplicate ids.

    Returns (psl, idx16, maskh):
      psl   [128, KS*NW*CAP] f16  multi-hot source->slot permute matrix
      idx16 [128, NW*CAP]   i16  slot -> column offset (or -1), replicated
      maskh [128, NWM*WIN]  f16  one-hot masks for the matmul windows
    """
    d = ids_b.astype(np.int64) - lo
    sel = (d >= 0) & (d < VH)
    psl = np.zeros((128, KS * NW * CAP), np.float16)
    idx16 = np.full(NW * CAP, -1, np.int16)
    maskh = np.zeros((128, NWM * WIN), np.float16)
    buckets = {}
    for s in np.nonzero(sel)[0]:
        v = int(d[s])
        buckets.setdefault((v // WIN, v % WIN), []).append(int(s))
    counts = np.zeros(NW, np.int64)
    for (wi, off), srcs in sorted(buckets.items()):
        i = counts[wi]
        assert i < CAP, f"window overflow in window {wi}"
        counts[wi] = i + 1
        idx16[wi * CAP + i] = off
        if wi < NWM:
            maskh[i, wi * WIN + off] = 1.0
        for s in srcs:
            psl[s % 128, (s // 128) * NW * CAP + wi * CAP + i] = 1.0
    idx_rep = np.ascontiguousarray(np.tile(idx16[None, :], (128, 1)))
    return psl, idx_rep, maskh


def kernel(**inputs: np.ndarray) -> np.ndarray:
    E = np.asarray(inputs["encoder_outputs"], dtype=np.float32)
    D = np.asarray(inputs["decoder_outputs"], dtype=np.float32)
    ids = np.asarray(inputs["inputs"]).astype(np.int64)
    ib = np.ascontiguousarray(np.asarray(inputs["input_bias"], dtype=np.float32))
    W = np.ascontiguousarray(np.asarray(inputs["W_proj"], dtype=np.float32))
    bp = np.ascontiguousarray(np.asarray(inputs["b_proj"], dtype=np.float32))
    ones = np.ones((1, 128), dtype=np.float32)

    nc = _get_nc()
    in_maps = []
    eT_b = [np.ascontiguousarray(E[b].T) for b in range(B)]
    dT_b = [np.ascontiguousarray(D[b].T) for b in range(B)]
    for c in range(N_CORES):
        b, vh = c // 2, c % 2
        psl, idx_rep, maskh = _prep_slots(ids[b], vh * VH)
        in_maps.append({
            "w": W,
            "eT": eT_b[b],
            "dT": dT_b[b],
            "bproj": bp,
            "sbias": ib[b],
            "ones": ones,
            "maskh": maskh,
            "pslh": psl,
            "idxh": idx_rep,
        })
    res = run_bass_kernel_spmd(nc, in_maps, core_ids=list(range(N_CORES)))
    out = np.empty((B, T, V), dtype=np.float32)
    for c in range(N_CORES):
        b, vh = c // 2, c % 2
        out[b, :, vh * VH:(vh + 1) * VH] = res.results[c]["out16"]
    return out


if __name__ == "__main__":
    nc = build_bass()
    print("built ok")
